# revision 5
# baseline (speedup 1.0000x reference)
"""Causal self-attention Bass kernel for Trainium2, 8-core data-parallel.

Problem: B=8, S=2048, C=256, H=4 heads, D=64. out = proj(causal_attn(qkv(x))).
Sharding: data-parallel over batch - one batch element per NeuronCore.

Per-core design (all matmul operands fp16 = full-rate PE, fp32 PSUM accum):
  - PE contracts over the partition dim: weights AND x are transposed on the
    host (free numpy repack + fp16 cast in kernel()), so no on-chip
    transposes; xT/wT stream in via DMA at kernel start.
  - qT, kT in [d, s] fp16 layout ([128, 2048] per head-pair: head0 rows
    0..63, head1 rows 64..127).
  - v stored per (s-tile, head) as [ones (64) | v_h (64)] 128-col fp16
    stationary blocks: the ones half makes attn@v emit sum(exp) on PSUM
    partitions 0..63 (base partition 0) for free, where the custom
    reciprocal_approx_fast op can read it without a repositioning copy.
  - scoresT [sk, sq] blocks, softmax WITHOUT max subtraction (|scores|/8 is
    small).  exp is SPLIT across engines by a cost-tracking balancer: DVE
    units run a one-instruction Schraudolph (fp16 bit trick into an int16
    tile: bits = round(1024*(log2e*0.125*s + 15 - 0.0565)), max rel err 3.9%,
    zero-mean; masked -NEG scores saturate on the f32->i16 convert to
    -0.0 = exact zero weight); ACT units run exact exp.  ex tiles are
    int16 holding fp16 bits: DVE writes natively, readers bitcast to fp16 (a
    write-side bitcast breaks on HW).  PSUM evacuation copies go through the
    same balancer so ACT and DVE loads stay even.
  - causal handling: interior sk-tiles full-width; diagonal sk-tiles compute
    only the valid column range, with the triangular boundary masked IN PSUM
    by an fp16 mask-matmul (UT(0/1).T @ LT(NEG)).  The mask matmul also
    CLOSES the accumulation group: a 64-row-stationary matmul with partial F
    and start&stop in one instruction corrupts/faults on HW.
  - attn@v accumulates a merged outT [sum|d, 2x512] tile (head0 cols 0:512,
    head1 cols 512:1024) over sk-tiles in PSUM; normalization = ONE
    reciprocal_approx_fast over [64,1024] + two DVE multiplies -> yT fp16.
  - proj: out = yT.T @ W_projT -> [s, c] -> DMA out.
  - PSUM: 3 x (2-bank G tiles) + 1 x (2-bank O tile) = 8 banks.  The
    3-deep G rotation breaks the scores(N+2)-waits-on-exp(N) WAR chain that
    paced the PE.
  - emission is driven as a single unit stream with a DEPTH-unit software
    pipeline skew (attn@v trails scores/exp, across block boundaries) plus
    interleaved qkv/proj prefetch items.
"""
import numpy as np

import concourse.bass as bass
import concourse.tile as tile
from concourse import bacc, mybir
from concourse.bass_utils import run_bass_kernel_spmd

dt = mybir.dt
F32 = dt.float32
F32R = dt.float32r
BF16 = dt.bfloat16
F16 = dt.float16
AF = mybir.ActivationFunctionType
ALU = mybir.AluOpType

S = 2048
C = 256
H = 4
D = 64
B = 8
ST = S // 128            # 16 s-tiles
SB = S // 512            # 4 sq-blocks of 512
GROUP = 2                # interior slots per exp group (2 PSUM banks)
DEPTH = 6               # attn@v trails scores/exp by this many units
EX_BUFS = 10
NEG = -57344.0
LOG2E = 1.4426950408889634
EXP_A = 1024.0 * LOG2E * 0.125
EXP_B = 1024.0 * (15.0 - 0.0565)


def _emit(nc, tc, ctx, xt, wa, wp, out, dbg=None, reps=1):
    const = ctx.enter_context(tc.tile_pool(name="const", bufs=1))
    per = ctx.enter_context(tc.tile_pool(name="persist", bufs=1))
    # unified PSUM pools: G 3x(2 banks) + O 1 buf x 2 banks = 8
    ps_g = ctx.enter_context(tc.tile_pool(name="ps_g", bufs=3, space="PSUM"))
    ps_o = ctx.enter_context(tc.tile_pool(name="ps_o", bufs=1, space="PSUM"))
    io_pool = ctx.enter_context(tc.tile_pool(name="io", bufs=8))
    ex_pool = ctx.enter_context(tc.tile_pool(name="expT", bufs=EX_BUFS))
    rc_pool = ctx.enter_context(tc.tile_pool(name="rc", bufs=3))
    out_pool = ctx.enter_context(tc.tile_pool(name="out_sb", bufs=2))

    def gtile(name):
        return ps_g.tile([128, GROUP * 512], F32, tag="G", name=name)

    def otile():
        return ps_o.tile([128, 1024], F32, tag="O", name="O")

    # ---- engine load balancer (ACT vs DVE) for exp + PSUM evacuations ----
    bal = {"A": 0.0, "D": 0.0}

    def _exp_cost(eng, w):
        return max(455.0, (w + 290) / 1.2) if eng == "A" else \
            max(462.0, (w + 187) / 0.96)

    def _copy_cost(eng, w):
        return _exp_cost(eng, w)

    def copy_bal(dst, src, w):
        """Evacuate [128, w] PSUM->SBUF on whichever engine is lighter;
        wide copies split at the balance point (128-col aligned)."""
        if w <= 512:
            ca, cd = _copy_cost("A", w), _copy_cost("D", w)
            if bal["A"] + ca <= bal["D"] + cd:
                nc.scalar.copy(dst[:, 0:w], src[:, 0:w])
                bal["A"] += ca
            else:
                nc.vector.tensor_copy(dst[:, 0:w], src[:, 0:w])
                bal["D"] += cd
            return
        # split: DVE takes f*w, ACT (1-f)*w, equalizing projected finish
        ra, rd = 1 / 1.2, 1 / 0.96
        f = (bal["A"] - bal["D"] + w * ra + 230 * ra - 60 * rd) / (w * (ra + rd))
        wd = int(round(f * w / 128.0)) * 128
        wd = max(0, min(w, wd))
        if wd > 0:
            nc.vector.tensor_copy(dst[:, 0:wd], src[:, 0:wd])
            bal["D"] += _copy_cost("D", wd)
        if wd < w:
            nc.scalar.copy(dst[:, wd:w], src[:, wd:w])
            bal["A"] += _copy_cost("A", w - wd)

    def copy_assign(dst, src, w):
        """Whole-tile assignment (for strided APs where splitting is awkward)."""
        ca, cd = _copy_cost("A", w), _copy_cost("D", w)
        if bal["A"] + ca <= bal["D"] + cd:
            nc.scalar.copy(dst, src)
            bal["A"] += ca
        else:
            nc.vector.tensor_copy(dst, src)
            bal["D"] += cd

    # ---- constants ----
    ut_bf = const.tile([128, 128], F16, tag="ut")        # ut[p,m] = 1 if m>=p
    nc.gpsimd.memset(ut_bf[:], 1.0)
    nc.gpsimd.affine_select(out=ut_bf[:], in_=ut_bf[:], compare_op=ALU.is_ge,
                            fill=0.0, base=0, pattern=[[1, 128]],
                            channel_multiplier=-1)
    lt_bf = const.tile([128, 128], F16, tag="lt")        # lt[m,g] = NEG if m>g
    nc.gpsimd.memset(lt_bf[:], NEG)
    nc.gpsimd.affine_select(out=lt_bf[:], in_=lt_bf[:], compare_op=ALU.is_gt,
                            fill=0.0, base=0, pattern=[[-1, 128]],
                            channel_multiplier=1)

    # ---- persistent SBUF tensors ----
    xT = [per.tile([128, S], F16, tag=f"xT{ci}", name=f"xT{ci}") for ci in range(2)]
    wT = [per.tile([128, 768], F16, tag=f"wT{ci}", name=f"wT{ci}") for ci in range(2)]
    wpT = [per.tile([128, 256], F16, tag=f"wpT{ci}", name=f"wpT{ci}") for ci in range(2)]
    qkT = [per.tile([128, S], F16, tag=f"qkT{ob}", name=f"qkT{ob}") for ob in range(4)]
    v_sb = per.tile([128, ST * H * 128], F16, tag="v")
    yT = [per.tile([128, S], F16, tag=f"yT{ci}", name=f"yT{ci}") for ci in range(2)]

    v4 = v_sb[:].rearrange("p (t h x) -> p t h x", t=ST, h=H)

    def emit_v_ones():
        v3m = v_sb[:].rearrange("p (c x) -> p c x", x=128)
        nc.gpsimd.memset(v3m[:, :, 0:64], 1.0)

    # ================= phase 1 =================
    # warm the exp table while the input DMAs run
    dummy = const.tile([1, 8], F32, tag="dummy")
    nc.scalar.activation(dummy[:], ut_bf[0:1, 0:8], AF.Exp, scale=1.0)

    def emit_load():
        for ci in range(2):
            nc.sync.dma_start(wT[ci][:], wa[ci * 128:(ci + 1) * 128, :])
        for ci in range(2):
            nc.sync.dma_start(wpT[ci][:], wp[ci * 128:(ci + 1) * 128, :])
        for half in range(2):
            for ci in range(2):
                nc.sync.dma_start(
                    xT[ci][:, half * 1024:(half + 1) * 1024],
                    xt[ci * 128:(ci + 1) * 128, half * 1024:(half + 1) * 1024])

    # qkv for one sq-block, split into three stream items (<=1024 cols each)
    def emit_qkv_a(sb):
        qg = gtile(f"qg{sb}")          # obs 0,1 (q both pairs)
        for ob in range(2):
            for ci in range(2):
                nc.tensor.matmul(qg[:, ob * 512:(ob + 1) * 512],
                                 wT[ci][:, ob * 128:(ob + 1) * 128],
                                 xT[ci][:, sb * 512:(sb + 1) * 512],
                                 start=(ci == 0), stop=(ci == 1))
        copy_bal(qkT[0][:, sb * 512:(sb + 1) * 512], qg[:, 0:512], 512)
        copy_bal(qkT[1][:, sb * 512:(sb + 1) * 512], qg[:, 512:1024], 512)

    def emit_qkv_b(sb):
        kg = gtile(f"kg{sb}")          # obs 2,3 (k both pairs)
        for i, ob in enumerate((2, 3)):
            for ci in range(2):
                nc.tensor.matmul(kg[:, i * 512:(i + 1) * 512],
                                 wT[ci][:, ob * 128:(ob + 1) * 128],
                                 xT[ci][:, sb * 512:(sb + 1) * 512],
                                 start=(ci == 0), stop=(ci == 1))
        copy_bal(qkT[2][:, sb * 512:(sb + 1) * 512], kg[:, 0:512], 512)
        copy_bal(qkT[3][:, sb * 512:(sb + 1) * 512], kg[:, 512:1024], 512)

    def emit_qkv_c(sb):
        # v for the 4 s-tiles of this sb
        vg = gtile(f"vg{sb}")
        for k in range(4):
            st = sb * 4 + k
            for ci in range(2):
                nc.tensor.matmul(vg[:, k * 256:(k + 1) * 256],
                                 xT[ci][:, st * 128:(st + 1) * 128],
                                 wT[ci][:, 512:768],
                                 start=(ci == 0), stop=(ci == 1))
        for k in range(4):
            st = sb * 4 + k
            copy_assign(v4[:, st, :, 64:128],
                        vg[:, k * 256:(k + 1) * 256]
                        .rearrange("p (h d) -> p h d", h=H), 256)

    def emit_qkv(sb):
        emit_qkv_a(sb)
        emit_qkv_b(sb)
        emit_qkv_c(sb)

    def emit_exp(ex, G, w):
        """exp(0.125*G) -> ex (an int16 tile holding fp16 bits).  The
        balancer sends units to DVE (Schraudolph fp16-bits; masked -NEG
        diag scores saturate on the int16 convert to -0.0, which adds
        exactly 0 downstream) or ACT (exact exp)."""
        ca, cd = _exp_cost("A", w), _exp_cost("D", w)
        if bal["D"] + cd <= bal["A"] + ca:
            nc.vector.tensor_scalar(ex[:, 0:w], G[:, 0:w], EXP_A, EXP_B,
                                    ALU.mult, ALU.add)
            bal["D"] += cd
        else:
            nc.scalar.activation(ex[:, 0:w].bitcast(F16), G[:, 0:w], AF.Exp,
                                 scale=0.125)
            bal["A"] += ca

    def attention_units(b, pair):
        """Yield (pre_fn, post_fn) work units for one (sq-block, head-pair).

        pre = scores matmuls + exp; post = attn@v (+ normalize on the last
        unit). The driver runs post DEPTH units behind pre, across block
        boundaries, so the scores/exp pipeline never drains.
        """
        qTp, kTp = qkT[pair], qkT[2 + pair]
        nt = 4 * b + 4               # sk-tiles for this sq-block
        O = [None]                   # allocated in the first post (attn@v)
        sq = slice(b * 512, (b + 1) * 512)

        units = []
        inner = [(t, h) for t in range(4 * b) for h in range(2)]
        for g0 in range(0, len(inner), GROUP):
            units.append(("int", inner[g0:g0 + GROUP]))
        # (j, h) -> packed col offset; bank-aligned, unit A spans 1024,
        # unit B exactly fills 1536 with no 512-boundary crossings
        units.append(("diag", [(0, 0, 0), (0, 1, 512)]))
        units.append(("diag", [(1, 0, 0), (3, 1, 384), (1, 1, 512),
                               (3, 0, 896)]))
        units.append(("diag", [(2, 0, 0), (2, 1, 256)]))

        def normalize():
            # [ones|v] stationary puts sums on partitions 0..63 = base
            # partition 0, which reciprocal_approx_fast requires; the
            # merged O tile lets one call cover both heads.
            rc = rc_pool.tile([64, 1024], F32, tag="rc", name="rc")
            nc.vector.reciprocal_approx_fast(rc[:], O[0][0:64, :])
            bal["D"] += _copy_cost("D", 1024)
            for h in range(2):
                nc.vector.tensor_tensor(yT[pair][h * 64:(h + 1) * 64, sq],
                                        O[0][64:128, h * 512:(h + 1) * 512],
                                        rc[:, h * 512:(h + 1) * 512],
                                        ALU.mult)
                bal["D"] += _copy_cost("D", 512) + 65

        for ui, (kind, payload) in enumerate(units):
            last = ui == len(units) - 1

            def pre(kind=kind, payload=payload):
                G = gtile("Ga")
                ex = ex_pool.tile([128, GROUP * 512], dt.int16, tag="ex",
                                  name="ex")

                def exf(c0, c1):
                    return ex[:, c0:c1].bitcast(F16)
                av = []
                if kind == "int":
                    for i, (t, h) in enumerate(payload):
                        hh = slice(h * 64, h * 64 + 64)
                        nc.tensor.matmul(G[:, i * 512:(i + 1) * 512],
                                         kTp[hh, t * 128:(t + 1) * 128],
                                         qTp[hh, sq], start=True, stop=True)
                        av.append((h, slice(0, 512),
                                   exf(i * 512, (i + 1) * 512), t))
                    emit_exp(ex, G, len(payload) * 512)
                else:
                    ext = 0
                    for j, h, c0 in payload:
                        t = 4 * b + j
                        off, w = j * 128, 512 - j * 128   # valid width
                        ext = max(ext, c0 + w)
                        hh = slice(h * 64, h * 64 + 64)
                        nc.tensor.matmul(G[:, c0:c0 + w],
                                         kTp[hh, t * 128:(t + 1) * 128],
                                         qTp[hh, b * 512 + off:(b + 1) * 512],
                                         start=True, stop=False,
                                         skip_group_check=True)
                        nc.tensor.matmul(G[:, c0:c0 + 128], ut_bf[:],
                                         lt_bf[:], start=False, stop=True,
                                         skip_group_check=True)
                        av.append((h, slice(off, 512), exf(c0, c0 + w), t))
                    emit_exp(ex, G, ext)
                return av

            def post(av, last=last, first=(ui == 0)):
                if first:
                    O[0] = otile()
                for h, osl, exsl, t in av:
                    nc.tensor.matmul(
                        O[0][:, h * 512 + osl.start:h * 512 + osl.stop],
                        v4[:, t, pair * 2 + h, :],
                        exsl, start=(t == 0), stop=(t == nt - 1),
                        skip_group_check=True)
                if last:
                    normalize()

            yield pre, post

    def emit_proj(b, ks=range(4)):
        pg = gtile(f"pg{b}")
        for k in ks:
            st = b * 4 + k
            for ci in range(2):
                nc.tensor.matmul(pg[:, k * 256:(k + 1) * 256],
                                 yT[ci][:, st * 128:(st + 1) * 128],
                                 wpT[ci][:], start=(ci == 0), stop=(ci == 1))
        k0 = ks[0] if isinstance(ks, list) else 0
        nk = len(list(ks))
        o_sb = out_pool.tile([128, nk * 256], F32, tag="o_sb", bufs=2)
        copy_bal(o_sb[:], pg[:, k0 * 256:(k0 + nk) * 256], nk * 256)
        nc.sync.dma_start(
            out[b * 512 + k0 * 128:b * 512 + (k0 + nk) * 128, :]
            .rearrange("(k p) c -> p k c", p=128),
            o_sb[:].rearrange("p (k c) -> p k c", k=nk))

    # drive: global DEPTH-unit skew over [attention | qkv | proj] streams.
    # proj(b) is placed one attention-unit into block b+1 so that b's last
    # normalize has already been emitted by the skewed driver.
    emit_v_ones()
    from collections import deque
    for _rep in range(reps):
        emit_load()
        stream = [("qkv", 0)]
        for sb in range(SB):
            units0 = [("attn", u) for u in attention_units(sb, 0)]
            stream.extend(units0)
            if sb + 1 < SB:
                stream.append(("qkvb", sb + 1))
            units1 = [("attn", u) for u in attention_units(sb, 1)]
            stream.extend(units1)
            if sb + 1 < SB:
                stream.insert(len(stream) - len(units1) + 1, ("qkva", sb + 1))
                stream.insert(len(stream) - len(units1) + 3, ("qkvc", sb + 1))
            if sb > 0:
                stream.insert(len(stream) - len(units1) + 2, ("proj", sb - 1))

        pending = deque()
        emitters = {"qkv": emit_qkv, "qkva": emit_qkv_a, "qkvb": emit_qkv_b,
                    "qkvc": emit_qkv_c, "proj": emit_proj}
        for kind, item in stream:
            if kind != "attn":
                emitters[kind](item)
                continue
            pre, post = item
            av = pre()
            if len(pending) >= DEPTH:
                av2, post2 = pending.popleft()
                post2(av2)
            pending.append((av, post))
        while pending:
            av2, post2 = pending.popleft()
            post2(av2)
        emit_proj(SB - 1, ks=[0, 1])
        emit_proj(SB - 1, ks=[2, 3])

    if dbg is not None:
        for ob in range(4):
            nc.sync.dma_start(dbg[f"qkT{ob}"], qkT[ob][:])
        nc.sync.dma_start(dbg["v"], v_sb[:])
        for ci in range(2):
            nc.sync.dma_start(dbg[f"yT{ci}"], yT[ci][:])


_CACHE = {}


def _build(debug=False, reps=1):
    key = ("nc_dbg" if debug else "nc") + (f"_r{reps}" if reps != 1 else "")
    if key in _CACHE:
        return _CACHE[key]
    from contextlib import ExitStack

    nc = bacc.Bacc("TRN2", target_bir_lowering=False, debug=False)
    xt = nc.dram_tensor("x_t", [C, S], F16, kind="ExternalInput").ap()
    wa = nc.dram_tensor("w_attn_t", [C, 3 * C], F16, kind="ExternalInput").ap()
    wp = nc.dram_tensor("w_proj_t", [C, C], F16, kind="ExternalInput").ap()
    out = nc.dram_tensor("out", [S, C], F32, kind="ExternalOutput").ap()
    dbg = None
    if debug:
        dbg = {}
        for ob in range(4):
            dbg[f"qkT{ob}"] = nc.dram_tensor(f"qkT{ob}", [128, S], F16, kind="ExternalOutput").ap()
        dbg["v"] = nc.dram_tensor("v", [128, ST * H * 128], F16, kind="ExternalOutput").ap()
        for ci in range(2):
            dbg[f"yT{ci}"] = nc.dram_tensor(f"yT{ci}", [128, S], F16, kind="ExternalOutput").ap()
    with tile.TileContext(nc) as tc, ExitStack() as ctx:
        _emit(nc, tc, ctx, xt, wa, wp, out, dbg, reps=reps)
    nc.compile()
    _CACHE[key] = nc
    return nc


def kernel(x, W_attn, W_proj):
    x = np.ascontiguousarray(np.asarray(x, dtype=np.float32))
    W_attn = np.ascontiguousarray(np.asarray(W_attn, dtype=np.float32))
    W_proj = np.ascontiguousarray(np.asarray(W_proj, dtype=np.float32))
    nc = _build()
    wat = np.ascontiguousarray(W_attn.T).astype(np.float16)
    wpt = np.ascontiguousarray(W_proj.T).astype(np.float16)
    xts = [np.ascontiguousarray(x[b].T).astype(np.float16) for b in range(B)]
    in_maps = [{"x_t": xts[b], "w_attn_t": wat, "w_proj_t": wpt}
               for b in range(B)]
    res = run_bass_kernel_spmd(nc, in_maps, core_ids=list(range(B)))
    return np.stack([res.results[b]["out"] for b in range(B)], axis=0)
